# revision 6
# baseline (speedup 1.0000x reference)
"""Trainium2 Bass kernel for nn_ExactSpectralHead (sparse resonance attention).

Reference computation (per batch element b):
    q = x @ Wq.T; k = x @ Wk.T; v = x @ Wv.T          # [T, H]
    s = (q @ k.T) * C**-0.5 + resonance_bias          # [T, T]
    s = where(allowed, s, -inf); p = softmax(s, -1)
    out = p @ v                                        # [T, H]

Strategy (8 NeuronCores, data-parallel over batch B=8, one b per core):
  - Host folds bias+mask into EB = exp(bias) * allowed (exact: exp(log1p(r)) = 1+r),
    so p_raw = exp(s_qk * scale) * EB with no -inf handling and exact zeros.
    Scores are bounded (|s|<~5), so no max-subtraction is needed; normalization
    (division by the row sum) is done on the HOST from the unnormalized PV
    output plus a row-sum computed on-device via a ones-matmul.
  - Everything is computed in a transposed layout so that every matmul contracts
    over the partition dim with zero on-device transposes:
      xT [C, T] (host-transposed), QT/KT = W.T^T @ xT -> [H, T],
      ST[tk, tq] = KT.T @ QT, PT = exp(ST*scale) * EBT,
      OT[h, tq] += V[tk,:].T @ PT[tk, tq]   (V in natural [T, H] layout),
      rowsum[tq] = ones.T @ sum_i PT_i, out = (OT / rowsum).T (host).
  - Q/K projections use fp8e4 inputs with DoubleRow matmuls (two 128-deep
    contraction chunks per pass -> 2x PE throughput). The score noise this
    introduces is ~0.3% absolute on s (scores are tiny vs the bias), well
    inside the 2e-2 tolerance. V stays bf16 (fp8 V noise would land ~1:1 on
    the output).
  - bf16 matmul inputs elsewhere (1 col/cycle on the PE), fp32 PSUM accum.
  - Causal block skipping: tiles with tk_chunk > tq_block are never touched.
  - The PE p-state ramps to 2.4GHz only after ~3us of continuous work, so a
    burst of dummy warmup matmuls runs during the initial DMA wait.
"""

import sys

sys.path.insert(0, "/opt/trn_rl_repo")

import numpy as np
import ml_dtypes

import concourse.bass as bass
import concourse.tile as tile
import concourse.mybir as mybir

# ----------------------------------------------------------------------------
# Workaround for walrus codegen "Too many sync wait commands" on the
# TileContext tail Drain: split the global-clock sem waits across multiple SP
# NOP instructions instead of attaching them all to the single Drain.
from concourse.vector_clock import ScopedClock, VectorClock


def _split_drain_and_barrier(self, tick_clock, wait_clock):
    """Cheap kernel tail: per-proc sem waits split across SP NOPs (walrus
    one-wait-per-instruction limit), then a regular-semaphore all-engine
    completion barrier (the stock EVSEM butterfly costs ~1.5-4us per hop),
    then GpSimd clears the tile semaphores. The next NEFF execution cannot
    start until every engine stream (including the clear) retires, so no
    trailing barrier is needed."""
    import concourse.mybir as _mybir

    nc = self.nc
    gc = tick_clock.global_clock
    n = len(gc)
    for p in range(n):
        t = gc[p]
        if t > 0:
            nop = nc.sync.nop(nofuse=True, hint=f"drain_wait_{p}")
            vc = VectorClock([t if i == p else 0 for i in range(n)])
            wait_clock.add_sem_waits(nop.ins, ScopedClock({None: vc}))

    tail_sem = nc.alloc_semaphore("tile_tail_sem")
    n_signals = 0
    for etype, eng in nc.engines.items():
        if etype == _mybir.EngineType.Pool:
            continue
        eng.drain(fusable=False)
        eng.sem_inc(tail_sem, 1)
        n_signals += 1
    nc.gpsimd.wait_ge(tail_sem, n_signals)
    assert self.sems is not None
    popped = nc._tile_sem_poison_stack.pop()
    assert popped is self._sem_poison
    nc.clear_and_free_semaphores(list(self.sems.allocated().values()))
    nc.gpsimd.sem_clear(range(tail_sem.num, tail_sem.num + 1))


tile.TileContext._drain_and_barrier = _split_drain_and_barrier
# ----------------------------------------------------------------------------

def _split_excess_waits(nc, max_waits=1):
    """Walrus codegen in this toolchain supports only one sem-wait per
    instruction; hoist excess waits onto preceding same-engine NOPs."""
    for f in nc.m.functions:
        for bb in f.blocks:
            new = []
            changed = False
            for inst in bb.instructions:
                if isinstance(inst, mybir.InstEventSemaphore):
                    # EventSemaphore ops measure ~3-5us on HW; their barrier
                    # semantics live entirely in sync_info (regular sems), so
                    # NoOps with the same sync_info are equivalent and fast.
                    # Waits and updates go on separate NoOps (wait first) to
                    # satisfy the no_semaphore_value_conflict ISA check.
                    si = inst.sync_info
                    changed = True
                    w = list(si.on_wait) if si else []
                    u = list(si.on_update) if si else []
                    if w:
                        new.append(
                            mybir.InstNoOp(
                                name=f"{inst.name}-wait",
                                engine=inst.engine,
                                bass_nofuse=True,
                                sync_info=mybir.SyncInfo(on_wait=w, on_update=[]),
                            )
                        )
                    new.append(
                        mybir.InstNoOp(
                            name=inst.name,
                            engine=inst.engine,
                            bass_nofuse=True,
                            sync_info=mybir.SyncInfo(on_wait=[], on_update=u),
                        )
                    )
                    continue
                si = inst.sync_info
                waits = list(si.on_wait) if si is not None else []
                if len(waits) > max_waits:
                    changed = True
                    excess, keep = waits[:-max_waits], waits[-max_waits:]
                    for k, w in enumerate(excess):
                        new.append(
                            mybir.InstNoOp(
                                name=f"{inst.name}-w{k}",
                                engine=inst.engine,
                                bass_nofuse=True,
                                sync_info=mybir.SyncInfo(on_wait=[w], on_update=[]),
                            )
                        )
                    inst.sync_info = mybir.SyncInfo(
                        on_wait=keep, on_update=list(si.on_update)
                    )
                new.append(inst)
            if changed:
                bb.instructions = new


B, T, C, H = 8, 2048, 1024, 128
NCORES = 8
SCALE = float(C) ** -0.5
P = 128
TQ = 512                 # tq block width (matmul moving dim)
NJ = T // TQ             # 4 tq blocks
NC_CHUNK = C // P        # 8 contraction chunks over channels
NCP = NC_CHUNK // 2      # 4 DoubleRow c-chunk pairs
NK = T // P              # 16 tk chunks
BF16 = mybir.dt.bfloat16
FP8 = mybir.dt.float8e4
F32 = mybir.dt.float32
DR = mybir.MatmulPerfMode.DoubleRow

_nc_cache = None


def _build_nc():
    nc = bass.Bass()
    # xT tiled: per (jt, half): contiguous 512KB, partition-major [p, 4c, q]
    xTt = nc.declare_dram_parameter("xTt", [NJ, 2, P, NC_CHUNK // 2, TQ], BF16, isOutput=False)
    # fp8 DoubleRow pack of xT: x8[j, cp, p, i, q] = xT[(2cp+i)*128+p, j*TQ+q]
    x8t = nc.declare_dram_parameter("x8t", [NJ, NCP, P, 2, TQ], FP8, isOutput=False)
    # fp8 DoubleRow packs of Wq/Wk: w8[cp, p, i, h] = W[h, (2cp+i)*128+p]
    w8q = nc.declare_dram_parameter("w8q", [NCP, P, 2, H], FP8, isOutput=False)
    w8k = nc.declare_dram_parameter("w8k", [NCP, P, 2, H], FP8, isOutput=False)
    wvT = nc.declare_dram_parameter("wvT", [C, H], BF16, isOutput=False)
    # ebT quad-tiled: [j, i4, 128, 4*TQ] with ebt[j,i4,p,k*TQ+q] = EB.T[128*(4*i4+k)+p, j*TQ+q]
    ebt = nc.declare_dram_parameter("ebt", [NJ, NK // 4, P, 4 * TQ], BF16, isOutput=False)
    # unnormalized PV output, bf16: [j, H, TQ] (host divides by rowsum + reassembles)
    outt = nc.declare_dram_parameter("outt", [NJ, H, TQ], BF16, isOutput=True)
    # rowsums, fp32: [j, 1, TQ]
    rsum = nc.declare_dram_parameter("rsum", [NJ, 1, TQ], F32, isOutput=True)

    wvT3 = wvT.rearrange("(o p) h -> p o h", p=P)

    with tile.TileContext(nc) as tc:
        with (
            tc.tile_pool(name="const", bufs=1) as const,
            tc.tile_pool(name="qkv_psum", bufs=2, space="PSUM") as qkv_psum,
            tc.tile_pool(name="st_psum", bufs=2, space="PSUM") as st_psum_pool,
            tc.tile_pool(name="ot_psum", bufs=1, space="PSUM") as ot_psum_pool,
            tc.tile_pool(name="rs_psum", bufs=1, space="PSUM") as rs_psum_pool,
            tc.tile_pool(name="pt", bufs=NK // 2 + 4) as pt_pool,
            tc.tile_pool(name="eb", bufs=10) as eb_pool,
            tc.tile_pool(name="outs", bufs=4) as out_pool,
        ):
            # ---------- persistent SBUF tensors ----------
            # warmup garbage tile: ramp the PE p-state while DMAs land.
            warm = const.tile([P, P], BF16, tag="warm", name="warm_sb")
            warm_ps = qkv_psum.tile([P, TQ], F32, tag="qkvps", name="qkvps")
            nc.vector.memset(warm[:], 0.0)
            for i in range(40):
                nc.tensor.matmul(
                    warm_ps[:, :P], lhsT=warm[:], rhs=warm[:],
                    start=True, stop=True, skip_group_check=True,
                )

            w8q_sb = const.tile([P, NCP, 2, H], FP8, tag="w8q", name="w8q_sb")
            w8k_sb = const.tile([P, NCP, 2, H], FP8, tag="w8k", name="w8k_sb")
            wv_sb = const.tile([P, NC_CHUNK, H], BF16, tag="wv", name="wv_sb")
            x8_sb = const.tile([P, NJ, NCP, 2, TQ], FP8, tag="x8", name="x8_sb")
            xT_sb = const.tile([P, NC_CHUNK, T], BF16, tag="xT", name="xT_sb")
            # all eb quads live in SBUF simultaneously (10 x 512KB bf16)
            ebqs = {}
            for j in range(NJ):
                for q4 in range(j + 1):
                    ebqs[(j, q4)] = eb_pool.tile(
                        [P, 4, TQ], BF16, tag="eb", name="eb"
                    )

            # ---------- all DMAs issued up-front, in consumption order, ----------
            # ---------- spread over the sync/scalar/gpsimd queues          ----------
            def load_x8(j, eng_a, eng_b):
                eng_a.dma_start(
                    x8_sb[:, j, 0:2], x8t[j, 0:2].rearrange("c p i q -> p c i q")
                )
                eng_b.dma_start(
                    x8_sb[:, j, 2:4], x8t[j, 2:4].rearrange("c p i q -> p c i q")
                )

            def load_xT(jt):
                for half in range(2):
                    eng = nc.sync if half == 0 else nc.scalar
                    eng.dma_start(
                        xT_sb[:, half * 4:(half + 1) * 4, jt * TQ:(jt + 1) * TQ],
                        xTt[jt, half],
                    )

            def load_eb(j, q4, eng):
                eng.dma_start(
                    ebqs[(j, q4)][:],
                    ebt[j, q4].rearrange("p (four q) -> p four q", four=4),
                )

            nc.gpsimd.dma_start(w8q_sb[:], w8q.rearrange("c p i h -> p c i h"))
            load_x8(0, nc.sync, nc.scalar)
            nc.gpsimd.dma_start(w8k_sb[:], w8k.rearrange("c p i h -> p c i h"))
            load_xT(0)
            nc.gpsimd.dma_start(wv_sb[:], wvT3[:])
            load_eb(0, 0, nc.gpsimd)
            load_x8(1, nc.sync, nc.scalar)
            load_xT(1)
            load_eb(1, 0, nc.gpsimd)
            load_eb(1, 1, nc.sync)
            load_x8(2, nc.scalar, nc.gpsimd)
            load_xT(2)
            load_eb(2, 0, nc.gpsimd)
            load_eb(2, 1, nc.sync)
            load_eb(2, 2, nc.scalar)
            load_x8(3, nc.gpsimd, nc.sync)
            load_xT(3)
            load_eb(3, 0, nc.scalar)
            load_eb(3, 1, nc.sync)
            load_eb(3, 2, nc.gpsimd)
            load_eb(3, 3, nc.scalar)

            QT_sb = const.tile([P, T], BF16, tag="QT", name="QT_sb")
            KT_sb = const.tile([P, T], BF16, tag="KT", name="KT_sb")
            v_sb = const.tile([P, NK, H], BF16, tag="V", name="v_sb")
            ones_sb = const.tile([P, P], BF16, tag="ones", name="ones_sb")
            nc.vector.memset(ones_sb[:], 1.0)

            # ---------- interleaved per tq-block: Q_j, K_j, V_(4j..4j+3), ATT_j ----------
            for j in range(NJ):
                # QT / KT for this block: fp8 DoubleRow over c-chunk pairs
                for w_sb, dst in ((w8q_sb, QT_sb), (w8k_sb, KT_sb)):
                    ps = qkv_psum.tile([P, TQ], F32, tag="qkvps", name="qkvps")
                    for cp in range(NCP):
                        nc.tensor.matmul(
                            ps[:],
                            lhsT=w_sb[:, cp],
                            rhs=x8_sb[:, j, cp],
                            start=(cp == 0),
                            stop=(cp == NCP - 1),
                            perf_mode=DR,
                        )
                    nc.vector.tensor_copy(dst[:, j * TQ:(j + 1) * TQ], ps[:])

                # V chunks 4j .. 4j+3 (bf16)
                for m in range(4 * j, 4 * j + 4):
                    ps = qkv_psum.tile([P, TQ], F32, tag="qkvps", name="qkvps")
                    for c in range(NC_CHUNK):
                        nc.tensor.matmul(
                            ps[:, :H],
                            lhsT=xT_sb[:, c, m * P:(m + 1) * P],
                            rhs=wv_sb[:, c, :],
                            start=(c == 0),
                            stop=(c == NC_CHUNK - 1),
                        )
                    nc.vector.tensor_copy(v_sb[:, m, :], ps[:, :H])

                # attention for tq block j (causal: tk chunks 0 .. 4j+3).
                # st pairs are emitted ahead; ot/rs matmuls for pair p are
                # emitted after st pair p+2, so the PE never waits on the
                # ACT(exp) -> DVE(mul) chase.
                n_i = 4 * j + 4
                n2 = n_i // 2
                ot = ot_psum_pool.tile([P, TQ], F32, tag="ot", name="ot")
                rs = rs_psum_pool.tile([P, TQ], F32, tag="rs", name="rs")
                pts = []

                def emit_ot(p):
                    for k in range(2):
                        i = 2 * p + k
                        nc.tensor.matmul(
                            ot[:],
                            lhsT=v_sb[:, i, :],
                            rhs=pts[p][:, k, :],
                            start=(i == 0),
                            stop=(i == n_i - 1),
                            skip_group_check=True,
                        )

                def emit_rs(p):
                    for k in range(2):
                        i = 2 * p + k
                        nc.tensor.matmul(
                            rs[:],
                            lhsT=ones_sb[:],
                            rhs=pts[p][:, k, :],
                            start=(i == 0),
                            stop=(i == n_i - 1),
                            skip_group_check=True,
                        )

                def emit_otrs(p):
                    emit_ot(p)
                    emit_rs(p)

                for p in range(n2):
                    st2 = st_psum_pool.tile([P, 2, TQ], F32, tag="st", name="st2")
                    for k in range(2):
                        i = 2 * p + k
                        nc.tensor.matmul(
                            st2[:, k, :],
                            lhsT=KT_sb[:, i * P:(i + 1) * P],
                            rhs=QT_sb[:, j * TQ:(j + 1) * TQ],
                            start=True,
                            stop=True,
                        )
                    pt = pt_pool.tile([P, 2, TQ], BF16, tag="pt", name="pt")
                    nc.scalar.activation(
                        pt[:], st2[:], mybir.ActivationFunctionType.Exp, scale=SCALE
                    )
                    nc.vector.tensor_mul(
                        pt[:], pt[:],
                        ebqs[(j, p // 2)][:, (p % 2) * 2:(p % 2) * 2 + 2, :],
                    )
                    pts.append(pt)
                    if p >= 2:
                        emit_otrs(p - 2)
                # trailing pairs: rowsum matmuls first so the rs DMA can go
                # out while the PE finishes the ot matmuls
                for p in range(max(0, n2 - 2), n2):
                    emit_rs(p)
                for p in range(max(0, n2 - 2), n2):
                    emit_ot(p)

                # rowsum out (all 128 psum partitions hold the same sums; DMA row 0)
                rsb = out_pool.tile([1, TQ], F32, tag="rsb", name="rsb")
                nc.vector.tensor_copy(rsb[:], rs[0:1, :])
                nc.gpsimd.dma_start(rsum[j][:], rsb[:])
                # unnormalized OT out, bf16, split in halves to pipeline
                HQ = TQ // 2
                otb = out_pool.tile([P, TQ], BF16, tag="otb", name="otb")
                for hh in range(2):
                    sl = slice(hh * HQ, (hh + 1) * HQ)
                    nc.vector.tensor_copy(otb[:, sl], ot[:, sl])
                    nc.scalar.dma_start(outt[j][:, sl], otb[:, sl])

    _split_excess_waits(nc)
    return nc


def _get_nc():
    global _nc_cache
    if _nc_cache is None:
        _nc_cache = _build_nc()
    return _nc_cache


def kernel(x, Wq, Wk, Wv, resonance_bias, allowed):
    x = np.asarray(x, dtype=np.float32)
    Wq = np.asarray(Wq, dtype=np.float32)
    Wk = np.asarray(Wk, dtype=np.float32)
    Wv = np.asarray(Wv, dtype=np.float32)
    resonance_bias = np.asarray(resonance_bias, dtype=np.float32)
    allowed = np.asarray(allowed)

    bf16 = ml_dtypes.bfloat16
    f8 = ml_dtypes.float8_e4m3
    eb = np.exp(resonance_bias) * allowed  # exp(log1p(r))*mask = (1+r)*mask, exact
    ebT = eb.T.astype(bf16)                              # [tk, tq]
    # quad-tiled: [j, i4, p, 4*TQ]
    ebt = np.ascontiguousarray(
        ebT.reshape(NK // 4, 4, P, NJ, TQ).transpose(3, 0, 2, 1, 4).reshape(
            NJ, NK // 4, P, 4 * TQ
        )
    )
    wvT = np.ascontiguousarray(Wv.T).astype(bf16)
    # fp8 DoubleRow weight packs: w8[cp, p, i, h] = W[h, (2cp+i)*128+p]
    w8q = np.ascontiguousarray(
        Wq.T.reshape(NCP, 2, P, H).transpose(0, 2, 1, 3)
    ).astype(f8)
    w8k = np.ascontiguousarray(
        Wk.T.reshape(NCP, 2, P, H).transpose(0, 2, 1, 3)
    ).astype(f8)

    in_maps = []
    for b in range(NCORES):
        xT = x[b].T  # [C, T] fp32
        xTt_b = np.ascontiguousarray(
            xT.astype(bf16)
            .reshape(2, NC_CHUNK // 2, P, NJ, TQ)
            .transpose(3, 0, 2, 1, 4)
        )
        # x8[j, cp, p, i, q] = xT[(2cp+i)*128+p, j*TQ+q]
        x8t_b = np.ascontiguousarray(
            xT.astype(f8)
            .reshape(NCP, 2, P, NJ, TQ)
            .transpose(3, 0, 2, 1, 4)
        )
        in_maps.append(
            {"xTt": xTt_b, "x8t": x8t_b, "w8q": w8q, "w8k": w8k,
             "wvT": wvT, "ebt": ebt}
        )

    nc = _get_nc()
    from concourse import bass2jax

    try:
        results = bass2jax.run_bass_via_pjrt(nc, in_maps, n_cores=NCORES)
    except Exception:
        # transient NRT execution errors occasionally wedge a core; one retry
        import time as _time

        _time.sleep(2.0)
        results = bass2jax.run_bass_via_pjrt(nc, in_maps, n_cores=NCORES)

    out = np.empty((B, T, H), dtype=np.float32)
    for b in range(NCORES):
        outt = results[b]["outt"].astype(np.float32)     # [NJ, H, TQ]
        rsum = results[b]["rsum"].astype(np.float32)     # [NJ, 1, TQ]
        norm = outt / rsum[:, None, 0, :]                # broadcast over H
        out[b] = norm.transpose(0, 2, 1).reshape(T, H)
    return out


# revision 13
# speedup vs baseline: 1.0701x; 1.0701x over previous
"""Trainium2 Bass kernel for nn_ExactSpectralHead (sparse resonance attention).

Reference computation (per batch element b):
    q = x @ Wq.T; k = x @ Wk.T; v = x @ Wv.T          # [T, H]
    s = (q @ k.T) * C**-0.5 + resonance_bias          # [T, T]
    s = where(allowed, s, -inf); p = softmax(s, -1)
    out = p @ v                                        # [T, H]

Strategy (8 NeuronCores, data-parallel over batch B=8, one b per core):
  - Host folds bias+mask into EB = exp(bias) * allowed (exact: exp(log1p(r)) = 1+r),
    so p_raw = exp(s_qk * scale) * EB with no -inf handling and exact zeros.
    Scores are bounded (|s|<~5), so no max-subtraction is needed; normalization
    (division by the row sum) is done on the HOST from the unnormalized PV
    output plus a row-sum computed on-device via a ones-matmul.
  - Everything is computed in a transposed layout so that every matmul contracts
    over the partition dim with zero on-device transposes:
      xT [C, T] (host-transposed), QT/KT = W.T^T @ xT -> [H, T],
      ST[tk, tq] = KT.T @ QT, PT = exp(ST*scale) * EBT,
      OT[h, tq] += V[tk,:].T @ PT[tk, tq]   (V in natural [T, H] layout),
      rowsum[tq] = ones.T @ sum_i PT_i, out = (OT / rowsum).T (host).
  - Q/K projections use fp8e4 inputs with DoubleRow matmuls (two 128-deep
    contraction chunks per pass -> 2x PE throughput). The score noise this
    introduces is ~0.3% absolute on s (scores are tiny vs the bias), well
    inside the 2e-2 tolerance. V stays bf16 (fp8 V noise would land ~1:1 on
    the output).
  - bf16 matmul inputs elsewhere (1 col/cycle on the PE), fp32 PSUM accum.
  - Causal block skipping: tiles with tk_chunk > tq_block are never touched.
  - The PE p-state ramps to 2.4GHz only after ~3us of continuous work, so a
    burst of dummy warmup matmuls runs during the initial DMA wait.
"""

import sys

sys.path.insert(0, "/opt/trn_rl_repo")

import numpy as np
import ml_dtypes

import concourse.bass as bass
import concourse.tile as tile
import concourse.mybir as mybir

# ----------------------------------------------------------------------------
# Workaround for walrus codegen "Too many sync wait commands" on the
# TileContext tail Drain: split the global-clock sem waits across multiple SP
# NOP instructions instead of attaching them all to the single Drain.
from concourse.vector_clock import ScopedClock, VectorClock


def _split_drain_and_barrier(self, tick_clock, wait_clock):
    """Cheap kernel tail: per-proc sem waits split across SP NOPs (walrus
    one-wait-per-instruction limit), then a regular-semaphore all-engine
    completion barrier (the stock EVSEM butterfly costs ~1.5-4us per hop),
    then GpSimd clears the tile semaphores. The next NEFF execution cannot
    start until every engine stream (including the clear) retires, so no
    trailing barrier is needed."""
    import concourse.mybir as _mybir

    nc = self.nc
    gc = tick_clock.global_clock
    n = len(gc)
    for p in range(n):
        t = gc[p]
        if t > 0:
            nop = nc.sync.nop(nofuse=True, hint=f"drain_wait_{p}")
            vc = VectorClock([t if i == p else 0 for i in range(n)])
            wait_clock.add_sem_waits(nop.ins, ScopedClock({None: vc}))

    tail_sem = nc.alloc_semaphore("tile_tail_sem")
    n_signals = 0
    for etype, eng in nc.engines.items():
        if etype == _mybir.EngineType.Pool:
            continue
        eng.drain(fusable=False)
        eng.sem_inc(tail_sem, 1)
        n_signals += 1
    nc.gpsimd.wait_ge(tail_sem, n_signals)
    assert self.sems is not None
    popped = nc._tile_sem_poison_stack.pop()
    assert popped is self._sem_poison
    nc.clear_and_free_semaphores(list(self.sems.allocated().values()))
    nc.gpsimd.sem_clear(range(tail_sem.num, tail_sem.num + 1))


tile.TileContext._drain_and_barrier = _split_drain_and_barrier
# ----------------------------------------------------------------------------

def _split_excess_waits(nc, max_waits=1):
    """Walrus codegen in this toolchain supports only one sem-wait per
    instruction; hoist excess waits onto preceding same-engine NOPs."""
    for f in nc.m.functions:
        for bb in f.blocks:
            new = []
            changed = False
            for inst in bb.instructions:
                if isinstance(inst, mybir.InstEventSemaphore):
                    # EventSemaphore ops measure ~3-5us on HW; their barrier
                    # semantics live entirely in sync_info (regular sems), so
                    # NoOps with the same sync_info are equivalent and fast.
                    # Waits and updates go on separate NoOps (wait first) to
                    # satisfy the no_semaphore_value_conflict ISA check.
                    si = inst.sync_info
                    changed = True
                    w = list(si.on_wait) if si else []
                    u = list(si.on_update) if si else []
                    if w:
                        new.append(
                            mybir.InstNoOp(
                                name=f"{inst.name}-wait",
                                engine=inst.engine,
                                bass_nofuse=True,
                                sync_info=mybir.SyncInfo(on_wait=w, on_update=[]),
                            )
                        )
                    new.append(
                        mybir.InstNoOp(
                            name=inst.name,
                            engine=inst.engine,
                            bass_nofuse=True,
                            sync_info=mybir.SyncInfo(on_wait=[], on_update=u),
                        )
                    )
                    continue
                si = inst.sync_info
                waits = list(si.on_wait) if si is not None else []
                if len(waits) > max_waits:
                    changed = True
                    excess, keep = waits[:-max_waits], waits[-max_waits:]
                    for k, w in enumerate(excess):
                        new.append(
                            mybir.InstNoOp(
                                name=f"{inst.name}-w{k}",
                                engine=inst.engine,
                                bass_nofuse=True,
                                sync_info=mybir.SyncInfo(on_wait=[w], on_update=[]),
                            )
                        )
                    inst.sync_info = mybir.SyncInfo(
                        on_wait=keep, on_update=list(si.on_update)
                    )
                new.append(inst)
            if changed:
                bb.instructions = new


B, T, C, H = 8, 2048, 1024, 128
NCORES = 8
SCALE = float(C) ** -0.5
P = 128
TQ = 512                 # tq block width (matmul moving dim)
NJ = T // TQ             # 4 tq blocks
NC_CHUNK = C // P        # 8 contraction chunks over channels
NCP = NC_CHUNK // 2      # 4 DoubleRow c-chunk pairs
NK = T // P              # 16 tk chunks
BF16 = mybir.dt.bfloat16
FP8 = mybir.dt.float8e4
F32 = mybir.dt.float32
DR = mybir.MatmulPerfMode.DoubleRow

_nc_cache = None


def _build_nc():
    nc = bass.Bass()
    # xT tiled: per (jt, half): contiguous 512KB, partition-major [p, 4c, q]
    xTt = nc.declare_dram_parameter("xTt", [NJ, 2, P, NC_CHUNK // 2, TQ], BF16, isOutput=False)
    # fp8 DoubleRow pack of xT: x8[j, p, cp, i, q] = xT[(2cp+i)*128+p, j*TQ+q]
    # (partition-major so each DMA moves 2-4KB contiguous per partition)
    x8t = nc.declare_dram_parameter("x8t", [NJ, P, NCP, 2, TQ], FP8, isOutput=False)
    # fp8 DoubleRow packs of Wq/Wk: w8[cp, p, i, h] = W[h, (2cp+i)*128+p]
    w8q = nc.declare_dram_parameter("w8q", [NCP, P, 2, H], FP8, isOutput=False)
    w8k = nc.declare_dram_parameter("w8k", [NCP, P, 2, H], FP8, isOutput=False)
    wvT = nc.declare_dram_parameter("wvT", [C, H], BF16, isOutput=False)
    # ebT quad-tiled: [j, i4, 128, 4*TQ] with ebt[j,i4,p,k*TQ+q] = EB.T[128*(4*i4+k)+p, j*TQ+q]
    # fp8: EB holds small integers (1+resonance <= 13), exactly representable
    ebt = nc.declare_dram_parameter("ebt", [NJ, NK // 4, P, 4 * TQ], FP8, isOutput=False)
    # unnormalized PV output, bf16: [j, H, TQ] (host divides by rowsum + reassembles)
    outt = nc.declare_dram_parameter("outt", [NJ, H, TQ], BF16, isOutput=True)
    # rowsums, fp32: [j, 1, TQ]
    rsum = nc.declare_dram_parameter("rsum", [NJ, 1, TQ], F32, isOutput=True)

    wvT3 = wvT.rearrange("(o p) h -> p o h", p=P)

    with tile.TileContext(nc) as tc:
        with (
            tc.tile_pool(name="const", bufs=1) as const,
            tc.tile_pool(name="qkv_psum", bufs=2, space="PSUM") as qkv_psum,
            tc.tile_pool(name="st_psum", bufs=2, space="PSUM") as st_psum_pool,
            tc.tile_pool(name="ot_psum", bufs=1, space="PSUM") as ot_psum_pool,
            tc.tile_pool(name="rs_psum", bufs=1, space="PSUM") as rs_psum_pool,
            tc.tile_pool(name="pt", bufs=NK // 2 + 4) as pt_pool,
            tc.tile_pool(name="eb", bufs=10) as eb_pool,
            tc.tile_pool(name="outs", bufs=4) as out_pool,
        ):
            # ---------- persistent SBUF tensors ----------
            # warmup garbage tile: ramp the PE p-state while DMAs land.
            warm = const.tile([P, P], BF16, tag="warm", name="warm_sb")
            warm_ps = qkv_psum.tile([P, TQ], F32, tag="qkvps", name="qkvps")
            nc.vector.memset(warm[:], 0.0)
            for i in range(40):
                nc.tensor.matmul(
                    warm_ps[:, :P], lhsT=warm[:], rhs=warm[:],
                    start=True, stop=True, skip_group_check=True,
                )

            w8q_sb = const.tile([P, NCP, 2, H], FP8, tag="w8q", name="w8q_sb")
            w8k_sb = const.tile([P, NCP, 2, H], FP8, tag="w8k", name="w8k_sb")
            wv_sb = const.tile([P, NC_CHUNK, H], BF16, tag="wv", name="wv_sb")
            x8_sb = const.tile([P, NJ, NCP, 2, TQ], FP8, tag="x8", name="x8_sb")
            xT_sb = const.tile([P, NC_CHUNK, T], BF16, tag="xT", name="xT_sb")
            # all eb quads live in SBUF simultaneously (10 x 256KB fp8)
            ebqs = {}
            for j in range(NJ):
                for q4 in range(j + 1):
                    ebqs[(j, q4)] = eb_pool.tile(
                        [P, 4, TQ], FP8, tag="eb", name="eb"
                    )

            # ---------- all DMAs issued up-front, in consumption order.   ----------
            # Queues: sync + gpsimd + vector (early only); scalar stays free
            # for the exp ACTIVATE stream. Per-queue lists are in global
            # consumption order so the round-robin DMA arbiter drains the
            # earliest-needed transfers first.
            def load_x8(j, eng_a, eng_b=None):
                if eng_b is None:
                    eng_a.dma_start(x8_sb[:, j], x8t[j])
                else:
                    eng_a.dma_start(x8_sb[:, j, 0:2], x8t[j, :, 0:2])
                    eng_b.dma_start(x8_sb[:, j, 2:4], x8t[j, :, 2:4])

            def load_xT(jt, eng_a, eng_b):
                for half, eng in ((0, eng_a), (1, eng_b)):
                    eng.dma_start(
                        xT_sb[:, half * 4:(half + 1) * 4, jt * TQ:(jt + 1) * TQ],
                        xTt[jt, half],
                    )

            def load_eb(j, q4, eng):
                eng.dma_start(
                    ebqs[(j, q4)][:],
                    ebt[j, q4].rearrange("p (four q) -> p four q", four=4),
                )

            nc.gpsimd.dma_start(w8q_sb[:], w8q.rearrange("c p i h -> p c i h"))
            load_x8(0, nc.sync, nc.scalar)
            nc.gpsimd.dma_start(w8k_sb[:], w8k.rearrange("c p i h -> p c i h"))
            load_xT(0, nc.sync, nc.scalar)
            nc.gpsimd.dma_start(wv_sb[:], wvT3[:])
            load_eb(0, 0, nc.sync)
            load_x8(1, nc.scalar)
            load_xT(1, nc.sync, nc.scalar)
            load_eb(1, 0, nc.gpsimd)
            load_eb(1, 1, nc.sync)
            load_x8(2, nc.gpsimd)
            load_xT(2, nc.sync, nc.scalar)
            load_eb(2, 0, nc.gpsimd)
            load_eb(2, 1, nc.sync)
            load_eb(2, 2, nc.gpsimd)
            load_x8(3, nc.sync)
            load_xT(3, nc.gpsimd, nc.scalar)
            load_eb(3, 0, nc.sync)
            load_eb(3, 1, nc.gpsimd)
            load_eb(3, 2, nc.sync)
            load_eb(3, 3, nc.gpsimd)

            QT_sb = const.tile([P, T], BF16, tag="QT", name="QT_sb")
            KT_sb = const.tile([P, T], BF16, tag="KT", name="KT_sb")
            v_sb = const.tile([P, NK, H], BF16, tag="V", name="v_sb")
            ones_sb = const.tile([P, P], BF16, tag="ones", name="ones_sb")
            nc.vector.memset(ones_sb[:], 1.0)

            # ---------- interleaved per tq-block: Q_j, K_j, V_(4j..4j+3), ATT_j ----------
            for j in range(NJ):
                # QT / KT for this block: fp8 DoubleRow over c-chunk pairs
                for w_sb, dst in ((w8q_sb, QT_sb), (w8k_sb, KT_sb)):
                    ps = qkv_psum.tile([P, TQ], F32, tag="qkvps", name="qkvps")
                    for cp in range(NCP):
                        nc.tensor.matmul(
                            ps[:],
                            lhsT=w_sb[:, cp],
                            rhs=x8_sb[:, j, cp],
                            start=(cp == 0),
                            stop=(cp == NCP - 1),
                            perf_mode=DR,
                        )
                    nc.vector.tensor_copy(dst[:, j * TQ:(j + 1) * TQ], ps[:])

                # V chunks 4j .. 4j+3 (bf16)
                for m in range(4 * j, 4 * j + 4):
                    ps = qkv_psum.tile([P, TQ], F32, tag="qkvps", name="qkvps")
                    for c in range(NC_CHUNK):
                        nc.tensor.matmul(
                            ps[:, :H],
                            lhsT=xT_sb[:, c, m * P:(m + 1) * P],
                            rhs=wv_sb[:, c, :],
                            start=(c == 0),
                            stop=(c == NC_CHUNK - 1),
                        )
                    nc.vector.tensor_copy(v_sb[:, m, :], ps[:, :H])

                # attention for tq block j (causal: tk chunks 0 .. 4j+3).
                # st pairs are emitted ahead; ot/rs matmuls for pair p are
                # emitted after st pair p+2, so the PE never waits on the
                # ACT(exp) -> DVE(mul) chase.
                n_i = 4 * j + 4
                n2 = n_i // 2
                ot = ot_psum_pool.tile([P, TQ], F32, tag="ot", name="ot")
                rs = rs_psum_pool.tile([P, TQ], F32, tag="rs", name="rs")
                pts = []

                def emit_ot(p):
                    for k in range(2):
                        i = 2 * p + k
                        nc.tensor.matmul(
                            ot[:],
                            lhsT=v_sb[:, i, :],
                            rhs=pts[p][:, k, :],
                            start=(i == 0),
                            stop=(i == n_i - 1),
                            skip_group_check=True,
                        )

                def emit_rs(p):
                    for k in range(2):
                        i = 2 * p + k
                        nc.tensor.matmul(
                            rs[:],
                            lhsT=ones_sb[:],
                            rhs=pts[p][:, k, :],
                            start=(i == 0),
                            stop=(i == n_i - 1),
                            skip_group_check=True,
                        )

                def emit_otrs(p):
                    emit_ot(p)
                    emit_rs(p)

                for p in range(n2):
                    st2 = st_psum_pool.tile([P, 2, TQ], F32, tag="st", name="st2")
                    for k in range(2):
                        i = 2 * p + k
                        nc.tensor.matmul(
                            st2[:, k, :],
                            lhsT=KT_sb[:, i * P:(i + 1) * P],
                            rhs=QT_sb[:, j * TQ:(j + 1) * TQ],
                            start=True,
                            stop=True,
                        )
                    pt = pt_pool.tile([P, 2, TQ], BF16, tag="pt", name="pt")
                    nc.scalar.activation(
                        pt[:], st2[:], mybir.ActivationFunctionType.Exp, scale=SCALE
                    )
                    nc.vector.tensor_mul(
                        pt[:], pt[:],
                        ebqs[(j, p // 2)][:, (p % 2) * 2:(p % 2) * 2 + 2, :],
                    )
                    pts.append(pt)
                    if p >= 2:
                        emit_otrs(p - 2)
                # trailing pairs: rowsum matmuls first so the rs DMA can go
                # out while the PE finishes the ot matmuls
                for p in range(max(0, n2 - 2), n2):
                    emit_rs(p)
                for p in range(max(0, n2 - 2), n2):
                    emit_ot(p)

                # rowsum out (all 128 psum partitions hold the same sums; DMA row 0)
                rsb = out_pool.tile([1, TQ], F32, tag="rsb", name="rsb")
                nc.vector.tensor_copy(rsb[:], rs[0:1, :])
                nc.gpsimd.dma_start(rsum[j][:], rsb[:])
                # unnormalized OT out, bf16, split in halves to pipeline
                HQ = TQ // 2
                otb = out_pool.tile([P, TQ], BF16, tag="otb", name="otb")
                for hh in range(2):
                    sl = slice(hh * HQ, (hh + 1) * HQ)
                    nc.vector.tensor_copy(otb[:, sl], ot[:, sl])
                    nc.gpsimd.dma_start(outt[j][:, sl], otb[:, sl])

    _split_excess_waits(nc)
    return nc


def _get_nc():
    global _nc_cache
    if _nc_cache is None:
        _nc_cache = _build_nc()
    return _nc_cache


def kernel(x, Wq, Wk, Wv, resonance_bias, allowed):
    x = np.asarray(x, dtype=np.float32)
    Wq = np.asarray(Wq, dtype=np.float32)
    Wk = np.asarray(Wk, dtype=np.float32)
    Wv = np.asarray(Wv, dtype=np.float32)
    resonance_bias = np.asarray(resonance_bias, dtype=np.float32)
    allowed = np.asarray(allowed)

    bf16 = ml_dtypes.bfloat16
    f8 = ml_dtypes.float8_e4m3
    eb = np.exp(resonance_bias) * allowed  # exp(log1p(r))*mask = (1+r)*mask, exact
    ebT = eb.T.astype(f8)                                # [tk, tq]; small ints, exact
    # quad-tiled: [j, i4, p, 4*TQ]
    ebt = np.ascontiguousarray(
        ebT.reshape(NK // 4, 4, P, NJ, TQ).transpose(3, 0, 2, 1, 4).reshape(
            NJ, NK // 4, P, 4 * TQ
        )
    )
    wvT = np.ascontiguousarray(Wv.T).astype(bf16)
    # fp8 DoubleRow weight packs: w8[cp, p, i, h] = W[h, (2cp+i)*128+p]
    w8q = np.ascontiguousarray(
        Wq.T.reshape(NCP, 2, P, H).transpose(0, 2, 1, 3)
    ).astype(f8)
    w8k = np.ascontiguousarray(
        Wk.T.reshape(NCP, 2, P, H).transpose(0, 2, 1, 3)
    ).astype(f8)

    in_maps = []
    for b in range(NCORES):
        xT = x[b].T  # [C, T] fp32
        xTt_b = np.ascontiguousarray(
            xT.astype(bf16)
            .reshape(2, NC_CHUNK // 2, P, NJ, TQ)
            .transpose(3, 0, 2, 1, 4)
        )
        # x8[j, p, cp, i, q] = xT[(2cp+i)*128+p, j*TQ+q]
        x8t_b = np.ascontiguousarray(
            xT.astype(f8)
            .reshape(NCP, 2, P, NJ, TQ)
            .transpose(3, 2, 0, 1, 4)
        )
        in_maps.append(
            {"xTt": xTt_b, "x8t": x8t_b, "w8q": w8q, "w8k": w8k,
             "wvT": wvT, "ebt": ebt}
        )

    nc = _get_nc()
    from concourse import bass2jax

    try:
        results = bass2jax.run_bass_via_pjrt(nc, in_maps, n_cores=NCORES)
    except Exception:
        # transient NRT execution errors occasionally wedge a core; one retry
        import time as _time

        _time.sleep(2.0)
        results = bass2jax.run_bass_via_pjrt(nc, in_maps, n_cores=NCORES)

    out = np.empty((B, T, H), dtype=np.float32)
    for b in range(NCORES):
        outt = results[b]["outt"].astype(np.float32)     # [NJ, H, TQ]
        rsum = results[b]["rsum"].astype(np.float32)     # [NJ, 1, TQ]
        norm = outt / rsum[:, None, 0, :]                # broadcast over H
        out[b] = norm.transpose(0, 2, 1).reshape(T, H)
    return out


# revision 18
# speedup vs baseline: 1.0813x; 1.0105x over previous
"""Trainium2 Bass kernel for nn_ExactSpectralHead (sparse resonance attention).

Reference computation (per batch element b):
    q = x @ Wq.T; k = x @ Wk.T; v = x @ Wv.T          # [T, H]
    s = (q @ k.T) * C**-0.5 + resonance_bias          # [T, T]
    s = where(allowed, s, -inf); p = softmax(s, -1)
    out = p @ v                                        # [T, H]

Strategy (8 NeuronCores, data-parallel over batch B=8, one b per core):
  - Host folds bias+mask into EB = exp(bias) * allowed (exact: exp(log1p(r)) = 1+r),
    so p_raw = exp(s_qk * scale) * EB with no -inf handling and exact zeros.
    Scores are bounded (|s|<~5), so no max-subtraction is needed; normalization
    (division by the row sum) is done on the HOST from the unnormalized PV
    output plus a row-sum computed on-device via a ones-matmul.
  - Everything is computed in a transposed layout so that every matmul contracts
    over the partition dim with zero on-device transposes:
      xT [C, T] (host-transposed), QT/KT = W.T^T @ xT -> [H, T],
      ST[tk, tq] = KT.T @ QT, PT = exp(ST*scale) * EBT,
      OT[h, tq] += V[tk,:].T @ PT[tk, tq]   (V in natural [T, H] layout),
      rowsum[tq] = ones.T @ sum_i PT_i, out = (OT / rowsum).T (host).
  - Q/K projections use fp8e4 inputs with DoubleRow matmuls (two 128-deep
    contraction chunks per pass -> 2x PE throughput). The score noise this
    introduces is ~0.3% absolute on s (scores are tiny vs the bias), well
    inside the 2e-2 tolerance. V stays bf16 (fp8 V noise would land ~1:1 on
    the output).
  - bf16 matmul inputs elsewhere (1 col/cycle on the PE), fp32 PSUM accum.
  - Causal block skipping: tiles with tk_chunk > tq_block are never touched.
  - The PE p-state ramps to 2.4GHz only after ~3us of continuous work, so a
    burst of dummy warmup matmuls runs during the initial DMA wait.
"""

import sys

sys.path.insert(0, "/opt/trn_rl_repo")

import numpy as np
import ml_dtypes

import concourse.bass as bass
import concourse.tile as tile
import concourse.mybir as mybir

# ----------------------------------------------------------------------------
# Workaround for walrus codegen "Too many sync wait commands" on the
# TileContext tail Drain: split the global-clock sem waits across multiple SP
# NOP instructions instead of attaching them all to the single Drain.
from concourse.vector_clock import ScopedClock, VectorClock


def _split_drain_and_barrier(self, tick_clock, wait_clock):
    """Cheap kernel tail: per-proc sem waits split across SP NOPs (walrus
    one-wait-per-instruction limit), then a regular-semaphore all-engine
    completion barrier (the stock EVSEM butterfly costs ~1.5-4us per hop),
    then GpSimd clears the tile semaphores. The next NEFF execution cannot
    start until every engine stream (including the clear) retires, so no
    trailing barrier is needed."""
    import concourse.mybir as _mybir

    nc = self.nc
    gc = tick_clock.global_clock
    n = len(gc)
    for p in range(n):
        t = gc[p]
        if t > 0:
            nop = nc.sync.nop(nofuse=True, hint=f"drain_wait_{p}")
            vc = VectorClock([t if i == p else 0 for i in range(n)])
            wait_clock.add_sem_waits(nop.ins, ScopedClock({None: vc}))

    tail_sem = nc.alloc_semaphore("tile_tail_sem")
    n_signals = 0
    for etype, eng in nc.engines.items():
        if etype == _mybir.EngineType.Pool:
            continue
        eng.drain(fusable=False)
        eng.sem_inc(tail_sem, 1)
        n_signals += 1
    nc.gpsimd.wait_ge(tail_sem, n_signals)
    assert self.sems is not None
    popped = nc._tile_sem_poison_stack.pop()
    assert popped is self._sem_poison
    nc.clear_and_free_semaphores(list(self.sems.allocated().values()))
    nc.gpsimd.sem_clear(range(tail_sem.num, tail_sem.num + 1))


tile.TileContext._drain_and_barrier = _split_drain_and_barrier
# ----------------------------------------------------------------------------

def _split_excess_waits(nc, max_waits=1):
    """Walrus codegen in this toolchain supports only one sem-wait per
    instruction; hoist excess waits onto preceding same-engine NOPs."""
    for f in nc.m.functions:
        for bb in f.blocks:
            new = []
            changed = False
            for inst in bb.instructions:
                if isinstance(inst, mybir.InstEventSemaphore):
                    # EventSemaphore ops measure ~3-5us on HW; their barrier
                    # semantics live entirely in sync_info (regular sems), so
                    # NoOps with the same sync_info are equivalent and fast.
                    # Waits and updates go on separate NoOps (wait first) to
                    # satisfy the no_semaphore_value_conflict ISA check.
                    si = inst.sync_info
                    changed = True
                    w = list(si.on_wait) if si else []
                    u = list(si.on_update) if si else []
                    if w:
                        new.append(
                            mybir.InstNoOp(
                                name=f"{inst.name}-wait",
                                engine=inst.engine,
                                bass_nofuse=True,
                                sync_info=mybir.SyncInfo(on_wait=w, on_update=[]),
                            )
                        )
                    new.append(
                        mybir.InstNoOp(
                            name=inst.name,
                            engine=inst.engine,
                            bass_nofuse=True,
                            sync_info=mybir.SyncInfo(on_wait=[], on_update=u),
                        )
                    )
                    continue
                si = inst.sync_info
                waits = list(si.on_wait) if si is not None else []
                if len(waits) > max_waits:
                    changed = True
                    excess, keep = waits[:-max_waits], waits[-max_waits:]
                    for k, w in enumerate(excess):
                        new.append(
                            mybir.InstNoOp(
                                name=f"{inst.name}-w{k}",
                                engine=inst.engine,
                                bass_nofuse=True,
                                sync_info=mybir.SyncInfo(on_wait=[w], on_update=[]),
                            )
                        )
                    inst.sync_info = mybir.SyncInfo(
                        on_wait=keep, on_update=list(si.on_update)
                    )
                new.append(inst)
            if changed:
                bb.instructions = new


B, T, C, H = 8, 2048, 1024, 128
NCORES = 8
SCALE = float(C) ** -0.5
P = 128
TQ = 512                 # tq block width (matmul moving dim)
NJ = T // TQ             # 4 tq blocks
NC_CHUNK = C // P        # 8 contraction chunks over channels
NCP = NC_CHUNK // 2      # 4 DoubleRow c-chunk pairs
NK = T // P              # 16 tk chunks
BF16 = mybir.dt.bfloat16
FP8 = mybir.dt.float8e4
F32 = mybir.dt.float32
DR = mybir.MatmulPerfMode.DoubleRow

_nc_cache = None


def _build_nc():
    nc = bass.Bass()
    # xT tiled: per (jt, half): contiguous 512KB, partition-major [p, 4c, q]
    xTt = nc.declare_dram_parameter("xTt", [NJ, 2, P, NC_CHUNK // 2, TQ], BF16, isOutput=False)
    # fp8 DoubleRow pack of xT: x8[j, p, cp, i, q] = xT[(2cp+i)*128+p, j*TQ+q]
    # (partition-major so each DMA moves 2-4KB contiguous per partition)
    x8t = nc.declare_dram_parameter("x8t", [NJ, P, NCP, 2, TQ], FP8, isOutput=False)
    # fp8 DoubleRow packs of Wq/Wk, partition-major: w8[p, cp, i, h] = W[h, (2cp+i)*128+p]
    w8q = nc.declare_dram_parameter("w8q", [P, NCP, 2, H], FP8, isOutput=False)
    w8k = nc.declare_dram_parameter("w8k", [P, NCP, 2, H], FP8, isOutput=False)
    # Wv pack, partition-major: wvp[p, c, h] = Wv[h, c*128+p]
    wvp = nc.declare_dram_parameter("wvp", [P, NC_CHUNK, H], BF16, isOutput=False)
    # ebT quad-tiled: [j, i4, 128, 4*TQ] with ebt[j,i4,p,k*TQ+q] = EB.T[128*(4*i4+k)+p, j*TQ+q]
    # fp8: EB holds small integers (1+resonance <= 13), exactly representable
    ebt = nc.declare_dram_parameter("ebt", [NJ, NK // 4, P, 4 * TQ], FP8, isOutput=False)
    # unnormalized PV output, bf16: [j, H, TQ] (host divides by rowsum + reassembles)
    outt = nc.declare_dram_parameter("outt", [NJ, H, TQ], BF16, isOutput=True)
    # rowsums, fp32: [j, 1, TQ]
    rsum = nc.declare_dram_parameter("rsum", [NJ, 1, TQ], F32, isOutput=True)

    with tile.TileContext(nc) as tc:
        with (
            tc.tile_pool(name="const", bufs=1) as const,
            tc.tile_pool(name="qkv_psum", bufs=2, space="PSUM") as qkv_psum,
            tc.tile_pool(name="st_psum", bufs=2, space="PSUM") as st_psum_pool,
            tc.tile_pool(name="ot_psum", bufs=1, space="PSUM") as ot_psum_pool,
            tc.tile_pool(name="rs_psum", bufs=1, space="PSUM") as rs_psum_pool,
            tc.tile_pool(name="pt", bufs=NK // 2 + 4) as pt_pool,
            tc.tile_pool(name="eb", bufs=10) as eb_pool,
            tc.tile_pool(name="outs", bufs=4) as out_pool,
        ):
            # ---------- persistent SBUF tensors ----------
            # warmup garbage tile: ramp the PE p-state while DMAs land.
            warm = const.tile([P, P], BF16, tag="warm", name="warm_sb")
            warm_ps = qkv_psum.tile([P, TQ], F32, tag="qkvps", name="qkvps")
            nc.vector.memset(warm[:], 0.0)
            for i in range(40):
                nc.tensor.matmul(
                    warm_ps[:, :P], lhsT=warm[:], rhs=warm[:],
                    start=True, stop=True, skip_group_check=True,
                )

            w8q_sb = const.tile([P, NCP, 2, H], FP8, tag="w8q", name="w8q_sb")
            w8k_sb = const.tile([P, NCP, 2, H], FP8, tag="w8k", name="w8k_sb")
            wv_sb = const.tile([P, NC_CHUNK, H], BF16, tag="wv", name="wv_sb")
            x8_sb = const.tile([P, NJ, NCP, 2, TQ], FP8, tag="x8", name="x8_sb")
            xT_sb = const.tile([P, NC_CHUNK, T], BF16, tag="xT", name="xT_sb")
            # all eb quads live in SBUF simultaneously (10 x 256KB fp8)
            ebqs = {}
            for j in range(NJ):
                for q4 in range(j + 1):
                    ebqs[(j, q4)] = eb_pool.tile(
                        [P, 4, TQ], FP8, tag="eb", name="eb"
                    )

            # ---------- all DMAs issued up-front, in consumption order.   ----------
            # Queues: sync + gpsimd + vector (early only); scalar stays free
            # for the exp ACTIVATE stream. Per-queue lists are in global
            # consumption order so the round-robin DMA arbiter drains the
            # earliest-needed transfers first.
            def load_x8(j, eng_a, eng_b=None):
                if eng_b is None:
                    eng_a.dma_start(x8_sb[:, j], x8t[j])
                else:
                    eng_a.dma_start(x8_sb[:, j, 0:2], x8t[j, :, 0:2])
                    eng_b.dma_start(x8_sb[:, j, 2:4], x8t[j, :, 2:4])

            def load_xT(jt, eng_a, eng_b):
                for half, eng in ((0, eng_a), (1, eng_b)):
                    eng.dma_start(
                        xT_sb[:, half * 4:(half + 1) * 4, jt * TQ:(jt + 1) * TQ],
                        xTt[jt, half],
                    )

            def load_eb(j, q4, eng):
                eng.dma_start(
                    ebqs[(j, q4)][:],
                    ebt[j, q4].rearrange("p (four q) -> p four q", four=4),
                )

            nc.gpsimd.dma_start(w8q_sb[:], w8q[:])
            load_x8(0, nc.sync, nc.scalar)
            nc.gpsimd.dma_start(w8k_sb[:], w8k[:])
            load_xT(0, nc.sync, nc.scalar)
            nc.gpsimd.dma_start(wv_sb[:], wvp[:])
            load_eb(0, 0, nc.sync)
            load_x8(1, nc.scalar)
            load_xT(1, nc.sync, nc.scalar)
            load_eb(1, 0, nc.gpsimd)
            load_eb(1, 1, nc.sync)
            load_x8(2, nc.gpsimd)
            load_xT(2, nc.sync, nc.scalar)
            load_eb(2, 0, nc.gpsimd)
            load_eb(2, 1, nc.sync)
            load_eb(2, 2, nc.gpsimd)
            load_x8(3, nc.sync)
            load_xT(3, nc.gpsimd, nc.scalar)
            load_eb(3, 0, nc.sync)
            load_eb(3, 1, nc.gpsimd)
            load_eb(3, 2, nc.sync)
            load_eb(3, 3, nc.gpsimd)

            QT_sb = const.tile([P, T], BF16, tag="QT", name="QT_sb")
            KT_sb = const.tile([P, T], BF16, tag="KT", name="KT_sb")
            v_sb = const.tile([P, NK, H], BF16, tag="V", name="v_sb")
            ones_sb = const.tile([P, P], BF16, tag="ones", name="ones_sb")
            nc.vector.memset(ones_sb[:], 1.0)

            # ---------- interleaved per tq-block: Q_j, K_j, V_(4j..4j+3), ATT_j ----------
            for j in range(NJ):
                # QT / KT for this block: fp8 DoubleRow over c-chunk pairs
                for w_sb, dst in ((w8q_sb, QT_sb), (w8k_sb, KT_sb)):
                    ps = qkv_psum.tile([P, TQ], F32, tag="qkvps", name="qkvps")
                    for cp in range(NCP):
                        nc.tensor.matmul(
                            ps[:],
                            lhsT=w_sb[:, cp],
                            rhs=x8_sb[:, j, cp],
                            start=(cp == 0),
                            stop=(cp == NCP - 1),
                            perf_mode=DR,
                        )
                    nc.vector.tensor_copy(dst[:, j * TQ:(j + 1) * TQ], ps[:])

                # V chunks 4j .. 4j+3 (bf16)
                for m in range(4 * j, 4 * j + 4):
                    ps = qkv_psum.tile([P, TQ], F32, tag="qkvps", name="qkvps")
                    for c in range(NC_CHUNK):
                        nc.tensor.matmul(
                            ps[:, :H],
                            lhsT=xT_sb[:, c, m * P:(m + 1) * P],
                            rhs=wv_sb[:, c, :],
                            start=(c == 0),
                            stop=(c == NC_CHUNK - 1),
                        )
                    nc.vector.tensor_copy(v_sb[:, m, :], ps[:, :H])

                # attention for tq block j (causal: tk chunks 0 .. 4j+3).
                # st pairs are emitted ahead; ot/rs matmuls for pair p are
                # emitted after st pair p+2, so the PE never waits on the
                # ACT(exp) -> DVE(mul) chase.
                n_i = 4 * j + 4
                n2 = n_i // 2
                ot = ot_psum_pool.tile([P, TQ], F32, tag="ot", name="ot")
                rs = rs_psum_pool.tile([P, TQ], F32, tag="rs", name="rs")
                pts = []

                def emit_ot(p):
                    for k in range(2):
                        i = 2 * p + k
                        nc.tensor.matmul(
                            ot[:],
                            lhsT=v_sb[:, i, :],
                            rhs=pts[p][:, k, :],
                            start=(i == 0),
                            stop=(i == n_i - 1),
                            skip_group_check=True,
                        )

                def emit_rs(p):
                    for k in range(2):
                        i = 2 * p + k
                        nc.tensor.matmul(
                            rs[:],
                            lhsT=ones_sb[:],
                            rhs=pts[p][:, k, :],
                            start=(i == 0),
                            stop=(i == n_i - 1),
                            skip_group_check=True,
                        )

                def emit_otrs(p):
                    emit_ot(p)
                    emit_rs(p)

                for p in range(n2):
                    st2 = st_psum_pool.tile([P, 2, TQ], F32, tag="st", name="st2")
                    for k in range(2):
                        i = 2 * p + k
                        nc.tensor.matmul(
                            st2[:, k, :],
                            lhsT=KT_sb[:, i * P:(i + 1) * P],
                            rhs=QT_sb[:, j * TQ:(j + 1) * TQ],
                            start=True,
                            stop=True,
                        )
                    pt = pt_pool.tile([P, 2, TQ], BF16, tag="pt", name="pt")
                    nc.scalar.activation(
                        pt[:], st2[:], mybir.ActivationFunctionType.Exp, scale=SCALE
                    )
                    nc.vector.tensor_mul(
                        pt[:], pt[:],
                        ebqs[(j, p // 2)][:, (p % 2) * 2:(p % 2) * 2 + 2, :],
                    )
                    pts.append(pt)
                    if p >= 2:
                        emit_otrs(p - 2)
                # trailing pairs: rowsum matmuls first so the rs DMA can go
                # out while the PE finishes the ot matmuls
                for p in range(max(0, n2 - 2), n2):
                    emit_rs(p)
                for p in range(max(0, n2 - 2), n2):
                    emit_ot(p)

                # rowsum out (all 128 psum partitions hold the same sums; DMA row 0)
                rsb = out_pool.tile([1, TQ], F32, tag="rsb", name="rsb")
                nc.vector.tensor_copy(rsb[:], rs[0:1, :])
                nc.gpsimd.dma_start(rsum[j][:], rsb[:])
                # unnormalized OT out, bf16, split in halves to pipeline
                HQ = TQ // 2
                otb = out_pool.tile([P, TQ], BF16, tag="otb", name="otb")
                for hh in range(2):
                    sl = slice(hh * HQ, (hh + 1) * HQ)
                    nc.vector.tensor_copy(otb[:, sl], ot[:, sl])
                    nc.gpsimd.dma_start(outt[j][:, sl], otb[:, sl])

    _split_excess_waits(nc)
    return nc


def _get_nc():
    global _nc_cache
    if _nc_cache is None:
        _nc_cache = _build_nc()
    return _nc_cache


def kernel(x, Wq, Wk, Wv, resonance_bias, allowed):
    x = np.asarray(x, dtype=np.float32)
    Wq = np.asarray(Wq, dtype=np.float32)
    Wk = np.asarray(Wk, dtype=np.float32)
    Wv = np.asarray(Wv, dtype=np.float32)
    resonance_bias = np.asarray(resonance_bias, dtype=np.float32)
    allowed = np.asarray(allowed)

    bf16 = ml_dtypes.bfloat16
    f8 = ml_dtypes.float8_e4m3
    eb = np.exp(resonance_bias) * allowed  # exp(log1p(r))*mask = (1+r)*mask, exact
    ebT = eb.T.astype(f8)                                # [tk, tq]; small ints, exact
    # quad-tiled: [j, i4, p, 4*TQ]
    ebt = np.ascontiguousarray(
        ebT.reshape(NK // 4, 4, P, NJ, TQ).transpose(3, 0, 2, 1, 4).reshape(
            NJ, NK // 4, P, 4 * TQ
        )
    )
    # Wv pack, partition-major: wvp[p, c, h] = Wv.T[c*128+p, h]
    wvp = np.ascontiguousarray(
        Wv.T.reshape(NC_CHUNK, P, H).transpose(1, 0, 2)
    ).astype(bf16)
    # fp8 DoubleRow weight packs, partition-major: w8[p, cp, i, h] = W[h, (2cp+i)*128+p]
    w8q = np.ascontiguousarray(
        Wq.T.reshape(NCP, 2, P, H).transpose(2, 0, 1, 3)
    ).astype(f8)
    w8k = np.ascontiguousarray(
        Wk.T.reshape(NCP, 2, P, H).transpose(2, 0, 1, 3)
    ).astype(f8)

    in_maps = []
    for b in range(NCORES):
        xT = x[b].T  # [C, T] fp32
        xTt_b = np.ascontiguousarray(
            xT.astype(bf16)
            .reshape(2, NC_CHUNK // 2, P, NJ, TQ)
            .transpose(3, 0, 2, 1, 4)
        )
        # x8[j, p, cp, i, q] = xT[(2cp+i)*128+p, j*TQ+q]
        x8t_b = np.ascontiguousarray(
            xT.astype(f8)
            .reshape(NCP, 2, P, NJ, TQ)
            .transpose(3, 2, 0, 1, 4)
        )
        in_maps.append(
            {"xTt": xTt_b, "x8t": x8t_b, "w8q": w8q, "w8k": w8k,
             "wvp": wvp, "ebt": ebt}
        )

    nc = _get_nc()
    from concourse import bass2jax

    try:
        results = bass2jax.run_bass_via_pjrt(nc, in_maps, n_cores=NCORES)
    except Exception:
        # transient NRT execution errors occasionally wedge a core; one retry
        import time as _time

        _time.sleep(2.0)
        results = bass2jax.run_bass_via_pjrt(nc, in_maps, n_cores=NCORES)

    out = np.empty((B, T, H), dtype=np.float32)
    for b in range(NCORES):
        outt = results[b]["outt"].astype(np.float32)     # [NJ, H, TQ]
        rsum = results[b]["rsum"].astype(np.float32)     # [NJ, 1, TQ]
        norm = outt / rsum[:, None, 0, :]                # broadcast over H
        out[b] = norm.transpose(0, 2, 1).reshape(T, H)
    return out


# revision 22
# speedup vs baseline: 1.1396x; 1.0539x over previous
"""Trainium2 Bass kernel for nn_ExactSpectralHead (sparse resonance attention).

Reference computation (per batch element b):
    q = x @ Wq.T; k = x @ Wk.T; v = x @ Wv.T          # [T, H]
    s = (q @ k.T) * C**-0.5 + resonance_bias          # [T, T]
    s = where(allowed, s, -inf); p = softmax(s, -1)
    out = p @ v                                        # [T, H]

Strategy (8 NeuronCores, data-parallel over batch B=8, one b per core):
  - Host folds bias+mask into EB = exp(bias) * allowed (exact: exp(log1p(r)) = 1+r),
    so p_raw = exp(s_qk * scale) * EB with no -inf handling and exact zeros.
    Scores are bounded (|s|<~5), so no max-subtraction is needed; normalization
    (division by the row sum) is done on the HOST from the unnormalized PV
    output plus a row-sum computed on-device via a ones-matmul.
  - Everything is computed in a transposed layout so that every matmul contracts
    over the partition dim with zero on-device transposes:
      xT [C, T] (host-transposed), QT/KT = W.T^T @ xT -> [H, T],
      ST[tk, tq] = KT.T @ QT, PT = exp(ST*scale) * EBT,
      OT[h, tq] += V[tk,:].T @ PT[tk, tq]   (V in natural [T, H] layout),
      rowsum[tq] = ones.T @ sum_i PT_i, out = (OT / rowsum).T (host).
  - Q/K projections use fp8e4 inputs with DoubleRow matmuls (two 128-deep
    contraction chunks per pass -> 2x PE throughput). The score noise this
    introduces is ~0.3% absolute on s (scores are tiny vs the bias), well
    inside the 2e-2 tolerance. V stays bf16 (fp8 V noise would land ~1:1 on
    the output).
  - bf16 matmul inputs elsewhere (1 col/cycle on the PE), fp32 PSUM accum.
  - Causal block skipping: tiles with tk_chunk > tq_block are never touched.
  - The PE p-state ramps to 2.4GHz only after ~3us of continuous work, so a
    burst of dummy warmup matmuls runs during the initial DMA wait.
"""

import sys

sys.path.insert(0, "/opt/trn_rl_repo")

import numpy as np
import ml_dtypes

import concourse.bass as bass
import concourse.tile as tile
import concourse.mybir as mybir

# ----------------------------------------------------------------------------
# Workaround for walrus codegen "Too many sync wait commands" on the
# TileContext tail Drain: split the global-clock sem waits across multiple SP
# NOP instructions instead of attaching them all to the single Drain.
from concourse.vector_clock import ScopedClock, VectorClock


def _split_drain_and_barrier(self, tick_clock, wait_clock):
    """Cheap kernel tail: per-proc sem waits split across SP NOPs (walrus
    one-wait-per-instruction limit), then a regular-semaphore all-engine
    completion barrier (the stock EVSEM butterfly costs ~1.5-4us per hop),
    then GpSimd clears the tile semaphores. The next NEFF execution cannot
    start until every engine stream (including the clear) retires, so no
    trailing barrier is needed."""
    import concourse.mybir as _mybir

    nc = self.nc
    gc = tick_clock.global_clock
    n = len(gc)
    for p in range(n):
        t = gc[p]
        if t > 0:
            nop = nc.sync.nop(nofuse=True, hint=f"drain_wait_{p}")
            vc = VectorClock([t if i == p else 0 for i in range(n)])
            wait_clock.add_sem_waits(nop.ins, ScopedClock({None: vc}))

    tail_sem = nc.alloc_semaphore("tile_tail_sem")
    n_signals = 0
    for etype, eng in nc.engines.items():
        if etype == _mybir.EngineType.Pool:
            continue
        eng.drain(fusable=False)
        eng.sem_inc(tail_sem, 1)
        n_signals += 1
    nc.gpsimd.wait_ge(tail_sem, n_signals)
    assert self.sems is not None
    popped = nc._tile_sem_poison_stack.pop()
    assert popped is self._sem_poison
    nc.clear_and_free_semaphores(list(self.sems.allocated().values()))
    nc.gpsimd.sem_clear(range(tail_sem.num, tail_sem.num + 1))


tile.TileContext._drain_and_barrier = _split_drain_and_barrier
# ----------------------------------------------------------------------------

def _split_excess_waits(nc, max_waits=1):
    """Walrus codegen in this toolchain supports only one sem-wait per
    instruction; hoist excess waits onto preceding same-engine NOPs."""
    for f in nc.m.functions:
        for bb in f.blocks:
            new = []
            changed = False
            for inst in bb.instructions:
                if isinstance(inst, mybir.InstEventSemaphore):
                    # EventSemaphore ops measure ~3-5us on HW; their barrier
                    # semantics live entirely in sync_info (regular sems), so
                    # NoOps with the same sync_info are equivalent and fast.
                    # Waits and updates go on separate NoOps (wait first) to
                    # satisfy the no_semaphore_value_conflict ISA check.
                    si = inst.sync_info
                    changed = True
                    w = list(si.on_wait) if si else []
                    u = list(si.on_update) if si else []
                    if w:
                        new.append(
                            mybir.InstNoOp(
                                name=f"{inst.name}-wait",
                                engine=inst.engine,
                                bass_nofuse=True,
                                sync_info=mybir.SyncInfo(on_wait=w, on_update=[]),
                            )
                        )
                    new.append(
                        mybir.InstNoOp(
                            name=inst.name,
                            engine=inst.engine,
                            bass_nofuse=True,
                            sync_info=mybir.SyncInfo(on_wait=[], on_update=u),
                        )
                    )
                    continue
                si = inst.sync_info
                waits = list(si.on_wait) if si is not None else []
                if len(waits) > max_waits:
                    changed = True
                    excess, keep = waits[:-max_waits], waits[-max_waits:]
                    for k, w in enumerate(excess):
                        new.append(
                            mybir.InstNoOp(
                                name=f"{inst.name}-w{k}",
                                engine=inst.engine,
                                bass_nofuse=True,
                                sync_info=mybir.SyncInfo(on_wait=[w], on_update=[]),
                            )
                        )
                    inst.sync_info = mybir.SyncInfo(
                        on_wait=keep, on_update=list(si.on_update)
                    )
                new.append(inst)
            if changed:
                bb.instructions = new


B, T, C, H = 8, 2048, 1024, 128
NCORES = 8
SCALE = float(C) ** -0.5
P = 128
TQ = 512                 # tq block width (matmul moving dim)
NJ = T // TQ             # 4 tq blocks
NC_CHUNK = C // P        # 8 contraction chunks over channels
NCP = NC_CHUNK // 2      # 4 DoubleRow c-chunk pairs
NK = T // P              # 16 tk chunks
BF16 = mybir.dt.bfloat16
FP8 = mybir.dt.float8e4
F32 = mybir.dt.float32
DR = mybir.MatmulPerfMode.DoubleRow

_nc_cache = None


def _build_nc():
    nc = bass.Bass()
    # xT tiled per tq-block, partition-major: one contiguous 1MB DMA per block
    xTt = nc.declare_dram_parameter("xTt", [NJ, P, NC_CHUNK, TQ], BF16, isOutput=False)
    # fp8 DoubleRow pack of xT: x8[j, p, cp, i, q] = xT[(2cp+i)*128+p, j*TQ+q]
    # (partition-major so each DMA moves 2-4KB contiguous per partition)
    x8t = nc.declare_dram_parameter("x8t", [NJ, P, NCP, 2, TQ], FP8, isOutput=False)
    # fp8 DoubleRow packs of Wq+Wk (one DMA), partition-major:
    # w8qk[p, s, cp, i, h] = W_s[h, (2cp+i)*128+p], s=0 -> Wq, s=1 -> Wk
    w8qk = nc.declare_dram_parameter("w8qk", [P, 2, NCP, 2, H], FP8, isOutput=False)
    # Wv pack, partition-major: wvp[p, c, h] = Wv[h, c*128+p]
    wvp = nc.declare_dram_parameter("wvp", [P, NC_CHUNK, H], BF16, isOutput=False)
    # ebT packed partition-major, j-grouped quads (quad (j,q4) at slot off_j+q4):
    # ebp[p, off_j + q4, k, q] = EB.T[128*(4*q4+k)+p, j*TQ+q]; one DMA per j.
    # fp8: EB holds small integers (1+resonance <= 13), exactly representable
    ebp = nc.declare_dram_parameter("ebp", [P, 10, 4, TQ], FP8, isOutput=False)
    # unnormalized PV output, bf16: [j, H, TQ] (host divides by rowsum + reassembles)
    outt = nc.declare_dram_parameter("outt", [NJ, H, TQ], BF16, isOutput=True)
    # rowsums, fp32: [j, 1, TQ]
    rsum = nc.declare_dram_parameter("rsum", [NJ, 1, TQ], F32, isOutput=True)

    with tile.TileContext(nc) as tc:
        with (
            tc.tile_pool(name="const", bufs=1) as const,
            tc.tile_pool(name="qkv_psum", bufs=2, space="PSUM") as qkv_psum,
            tc.tile_pool(name="st_psum", bufs=2, space="PSUM") as st_psum_pool,
            tc.tile_pool(name="ot_psum", bufs=1, space="PSUM") as ot_psum_pool,
            tc.tile_pool(name="rs_psum", bufs=1, space="PSUM") as rs_psum_pool,
            tc.tile_pool(name="pt", bufs=NK // 2 + 4) as pt_pool,
            tc.tile_pool(name="eb", bufs=1) as eb_pool,
            tc.tile_pool(name="outs", bufs=4) as out_pool,
        ):
            # ---------- persistent SBUF tensors ----------
            # warmup garbage tile: ramp the PE p-state while DMAs land.
            warm = const.tile([P, P], BF16, tag="warm", name="warm_sb")
            warm_ps = qkv_psum.tile([P, TQ], F32, tag="qkvps", name="qkvps")
            nc.vector.memset(warm[:], 0.0)
            for i in range(40):
                nc.tensor.matmul(
                    warm_ps[:, :P], lhsT=warm[:], rhs=warm[:],
                    start=True, stop=True, skip_group_check=True,
                )

            w8qk_sb = const.tile([P, 2, NCP, 2, H], FP8, tag="w8qk", name="w8qk_sb")
            wv_sb = const.tile([P, NC_CHUNK, H], BF16, tag="wv", name="wv_sb")
            x8_sb = const.tile([P, NJ, NCP, 2, TQ], FP8, tag="x8", name="x8_sb")
            xT_sb = const.tile([P, NC_CHUNK, T], BF16, tag="xT", name="xT_sb")
            # per-j eb tiles (all quads of a block in one tile, one DMA)
            ebjs = [
                eb_pool.tile([P, (j + 1) * 4, TQ], FP8, tag=f"eb{j}", name=f"eb{j}")
                for j in range(NJ)
            ]
            EBOFF = [0, 1, 3, 6]

            # ---------- all DMAs issued up-front, in consumption order.   ----------
            # Few, large transfers: the DMA queues have a multi-us fixed cost
            # per transfer. Per-queue lists are in global consumption order;
            # start-critical bulk rides the fast sync/gpsimd queues, the slow
            # scalar queue gets only mid/late items.
            def load_x8(j, eng):
                eng.dma_start(x8_sb[:, j], x8t[j])

            def load_xT(jt, eng):
                eng.dma_start(xT_sb[:, :, jt * TQ:(jt + 1) * TQ], xTt[jt])

            def load_eb(j, eng):
                eng.dma_start(ebjs[j][:], ebp[:, EBOFF[j]:EBOFF[j] + j + 1])

            nc.gpsimd.dma_start(w8qk_sb[:], w8qk[:])    # 256KB
            load_x8(0, nc.sync)                         # 512KB
            nc.gpsimd.dma_start(wv_sb[:], wvp[:])       # 256KB
            load_xT(0, nc.scalar)                       # 1MB (slow q, but 1st)
            load_eb(0, nc.gpsimd)                       # 256KB
            load_x8(1, nc.sync)                         # 512KB
            load_xT(1, nc.gpsimd)                       # 1MB
            load_eb(1, nc.sync)                         # 512KB
            load_x8(2, nc.scalar)                       # 512KB
            load_xT(2, nc.sync)                         # 1MB
            load_eb(2, nc.gpsimd)                       # 768KB
            load_x8(3, nc.scalar)                       # 512KB
            load_xT(3, nc.sync)                         # 1MB
            load_eb(3, nc.gpsimd)                       # 1MB

            QT_sb = const.tile([P, T], BF16, tag="QT", name="QT_sb")
            KT_sb = const.tile([P, T], BF16, tag="KT", name="KT_sb")
            v_sb = const.tile([P, NK, H], BF16, tag="V", name="v_sb")
            ones_sb = const.tile([P, P], BF16, tag="ones", name="ones_sb")
            nc.vector.memset(ones_sb[:], 1.0)

            # ---------- interleaved per tq-block: Q_j, K_j, V_(4j..4j+3), ATT_j ----------
            for j in range(NJ):
                # QT / KT for this block: fp8 DoubleRow over c-chunk pairs
                for s, dst in ((0, QT_sb), (1, KT_sb)):
                    ps = qkv_psum.tile([P, TQ], F32, tag="qkvps", name="qkvps")
                    for cp in range(NCP):
                        nc.tensor.matmul(
                            ps[:],
                            lhsT=w8qk_sb[:, s, cp],
                            rhs=x8_sb[:, j, cp],
                            start=(cp == 0),
                            stop=(cp == NCP - 1),
                            perf_mode=DR,
                        )
                    nc.vector.tensor_copy(dst[:, j * TQ:(j + 1) * TQ], ps[:])

                # V chunks 4j .. 4j+3 (bf16)
                for m in range(4 * j, 4 * j + 4):
                    ps = qkv_psum.tile([P, TQ], F32, tag="qkvps", name="qkvps")
                    for c in range(NC_CHUNK):
                        nc.tensor.matmul(
                            ps[:, :H],
                            lhsT=xT_sb[:, c, m * P:(m + 1) * P],
                            rhs=wv_sb[:, c, :],
                            start=(c == 0),
                            stop=(c == NC_CHUNK - 1),
                        )
                    nc.vector.tensor_copy(v_sb[:, m, :], ps[:, :H])

                # attention for tq block j (causal: tk chunks 0 .. 4j+3).
                # st pairs are emitted ahead; ot/rs matmuls for pair p are
                # emitted after st pair p+2, so the PE never waits on the
                # ACT(exp) -> DVE(mul) chase.
                n_i = 4 * j + 4
                n2 = n_i // 2
                ot = ot_psum_pool.tile([P, TQ], F32, tag="ot", name="ot")
                rs = rs_psum_pool.tile([P, TQ], F32, tag="rs", name="rs")
                pts = []

                def emit_ot(p):
                    for k in range(2):
                        i = 2 * p + k
                        nc.tensor.matmul(
                            ot[:],
                            lhsT=v_sb[:, i, :],
                            rhs=pts[p][:, k, :],
                            start=(i == 0),
                            stop=(i == n_i - 1),
                            skip_group_check=True,
                        )

                def emit_rs(p):
                    for k in range(2):
                        i = 2 * p + k
                        nc.tensor.matmul(
                            rs[:],
                            lhsT=ones_sb[:],
                            rhs=pts[p][:, k, :],
                            start=(i == 0),
                            stop=(i == n_i - 1),
                            skip_group_check=True,
                        )

                def emit_otrs(p):
                    emit_ot(p)
                    emit_rs(p)

                for p in range(n2):
                    st2 = st_psum_pool.tile([P, 2, TQ], F32, tag="st", name="st2")
                    for k in range(2):
                        i = 2 * p + k
                        nc.tensor.matmul(
                            st2[:, k, :],
                            lhsT=KT_sb[:, i * P:(i + 1) * P],
                            rhs=QT_sb[:, j * TQ:(j + 1) * TQ],
                            start=True,
                            stop=True,
                        )
                    pt = pt_pool.tile([P, 2, TQ], BF16, tag="pt", name="pt")
                    nc.scalar.activation(
                        pt[:], st2[:], mybir.ActivationFunctionType.Exp, scale=SCALE
                    )
                    nc.vector.tensor_mul(
                        pt[:], pt[:], ebjs[j][:, 2 * p:2 * p + 2, :]
                    )
                    pts.append(pt)
                    if p >= 2:
                        emit_otrs(p - 2)
                # trailing pairs: rowsum matmuls first so the rs DMA can go
                # out while the PE finishes the ot matmuls
                for p in range(max(0, n2 - 2), n2):
                    emit_rs(p)
                for p in range(max(0, n2 - 2), n2):
                    emit_ot(p)

                # rowsum out (all 128 psum partitions hold the same sums; DMA row 0)
                rsb = out_pool.tile([1, TQ], F32, tag="rsb", name="rsb")
                nc.vector.tensor_copy(rsb[:], rs[0:1, :])
                nc.gpsimd.dma_start(rsum[j][:], rsb[:])
                # unnormalized OT out, bf16, one DMA per block
                otb = out_pool.tile([P, TQ], BF16, tag="otb", name="otb")
                nc.vector.tensor_copy(otb[:], ot[:])
                nc.scalar.dma_start(outt[j][:], otb[:])

    _split_excess_waits(nc)
    return nc


def _get_nc():
    global _nc_cache
    if _nc_cache is None:
        _nc_cache = _build_nc()
    return _nc_cache


def kernel(x, Wq, Wk, Wv, resonance_bias, allowed):
    x = np.asarray(x, dtype=np.float32)
    Wq = np.asarray(Wq, dtype=np.float32)
    Wk = np.asarray(Wk, dtype=np.float32)
    Wv = np.asarray(Wv, dtype=np.float32)
    resonance_bias = np.asarray(resonance_bias, dtype=np.float32)
    allowed = np.asarray(allowed)

    bf16 = ml_dtypes.bfloat16
    f8 = ml_dtypes.float8_e4m3
    eb = np.exp(resonance_bias) * allowed  # exp(log1p(r))*mask = (1+r)*mask, exact
    ebT = eb.T.astype(f8)                                # [tk, tq]; small ints, exact
    # partition-major, j-grouped quads: ebp[p, EBOFF[j]+q4, k, q]
    #   = EB.T[128*(4*q4+k)+p, j*TQ+q]
    ebq5 = ebT.reshape(NK // 4, 4, P, NJ, TQ)            # [q4, k, p, j, q]
    ebp = np.empty((P, 10, 4, TQ), dtype=f8)
    EBOFF = [0, 1, 3, 6]
    for j in range(NJ):
        for q4 in range(j + 1):
            ebp[:, EBOFF[j] + q4] = ebq5[q4, :, :, j, :].transpose(1, 0, 2)
    ebp = np.ascontiguousarray(ebp)
    # Wv pack, partition-major: wvp[p, c, h] = Wv.T[c*128+p, h]
    wvp = np.ascontiguousarray(
        Wv.T.reshape(NC_CHUNK, P, H).transpose(1, 0, 2)
    ).astype(bf16)
    # fp8 DoubleRow weight packs, partition-major, q+k stacked:
    # w8qk[p, s, cp, i, h] = W_s[h, (2cp+i)*128+p]
    w8qk = np.ascontiguousarray(
        np.stack(
            [W.T.reshape(NCP, 2, P, H).transpose(2, 0, 1, 3) for W in (Wq, Wk)],
            axis=1,
        )
    ).astype(f8)

    in_maps = []
    for b in range(NCORES):
        xT = x[b].T  # [C, T] fp32
        # xTt[j, p, c, q] = xT[c*128+p, j*TQ+q]
        xTt_b = np.ascontiguousarray(
            xT.astype(bf16)
            .reshape(NC_CHUNK, P, NJ, TQ)
            .transpose(2, 1, 0, 3)
        )
        # x8[j, p, cp, i, q] = xT[(2cp+i)*128+p, j*TQ+q]
        x8t_b = np.ascontiguousarray(
            xT.astype(f8)
            .reshape(NCP, 2, P, NJ, TQ)
            .transpose(3, 2, 0, 1, 4)
        )
        in_maps.append(
            {"xTt": xTt_b, "x8t": x8t_b, "w8qk": w8qk, "wvp": wvp, "ebp": ebp}
        )

    nc = _get_nc()
    from concourse import bass2jax

    try:
        results = bass2jax.run_bass_via_pjrt(nc, in_maps, n_cores=NCORES)
    except Exception:
        # transient NRT execution errors occasionally wedge a core; one retry
        import time as _time

        _time.sleep(2.0)
        results = bass2jax.run_bass_via_pjrt(nc, in_maps, n_cores=NCORES)

    out = np.empty((B, T, H), dtype=np.float32)
    for b in range(NCORES):
        outt = results[b]["outt"].astype(np.float32)     # [NJ, H, TQ]
        rsum = results[b]["rsum"].astype(np.float32)     # [NJ, 1, TQ]
        norm = outt / rsum[:, None, 0, :]                # broadcast over H
        out[b] = norm.transpose(0, 2, 1).reshape(T, H)
    return out
